# revision 1
# baseline (speedup 1.0000x reference)
"""AttGNN kernel for 8 Trainium2 NeuronCores (Bass/Tile).

Math (reference):
    sup2 = sup + I
    h    = feat @ W_map                      [N, 64]
    s    = h @ U ; t = h @ V                 [N, 1]
    att  = softmax_rows(mask(tanh(s_i + t_j + b), sup2[j, i] > 0))   [N, N]
    gat  = tanh(att @ h)                     [N, 64]
    out  = normalize_rows(relu((sup2 @ gat) @ W_gcn))                [N, 64]

Distribution: 1D row-shard of sup/att over 8 cores (1024 rows each).
Both the attention mask and the two big matmuls need sup2 with the
*global* node index on SBUF partitions, i.e. the transpose of the shard
(T[c, j'] = sup2[shard j', c]).  The per-core input buffer is marshalled
host-side in that layout (input prep), and the device runs a single
pipelined fp32->bf16 cast-load stream of it.

Per core (c = global node index, 64 tiles of 128; j' = local shard row):
  phase 1:  n[c, j'] = (T > 0) * exp(tanh(s_c + t_j' + b))      (unnormalised
            masked attention weights, transposed layout)
            unnorm[65, j'] = sum_c [h | 1][c, :]^T n[c, j']     (PE, PSUM acc)
            row 64 is the softmax denominator d[j'].
            gat[j', :] = tanh(unnorm[0:64, j'] / d[j'])
  all-gather gat (bf16) -> full [8192, 64]
  phase 2:  M[d, i'] = sum_j gat[j, d] T[j, i']                 (PE)
            pre[e, i'] = W_gcn^T M                               (PE)
            out[i', :] = normalize(relu(pre))^T                  -> store
"""

import os
import numpy as np

N = 8192
DIN = 128
DG = 64
M_CORES = 8
S = N // M_CORES          # 1024 shard rows per core
P = 128                   # partitions
NCT = N // P              # 64 c-tiles
NPAIR = NCT // 2          # 32 pairs of c-tiles
F2 = 2 * S                # 2048 free elems per pair tile

_built = {}


def _build(reps=1):
    skip_tail = bool(int(os.environ.get("K_SKIP_TAIL", "0")))
    skip_p2 = bool(int(os.environ.get("K_SKIP_P2", "0")))
    split_ag = bool(int(os.environ.get("K_SPLIT_AG", "0")))
    use_ts_tt = bool(int(os.environ.get("K_TS_TT", "0")))
    skip_main = bool(int(os.environ.get("K_SKIP_MAIN", "0")))
    import concourse.bass as bass
    import concourse.bacc as bacc
    import concourse.mybir as mybir
    import concourse.tile as tile
    from concourse.masks import make_identity

    f32 = mybir.dt.float32
    bf16 = mybir.dt.bfloat16
    Alu = mybir.AluOpType
    Act = mybir.ActivationFunctionType

    nc = bacc.Bacc(None)

    supT = nc.declare_dram_parameter("supT", [N, S], f32, isOutput=False)
    featT = nc.declare_dram_parameter("featT", [DIN, N], f32, isOutput=False)
    featTs = nc.declare_dram_parameter("featTs", [DIN, S], f32, isOutput=False)
    W_map = nc.declare_dram_parameter("W_map", [DIN, DG], f32, isOutput=False)
    U_in = nc.declare_dram_parameter("U", [DG, 1], f32, isOutput=False)
    V_in = nc.declare_dram_parameter("V", [DG, 1], f32, isOutput=False)
    b_in = nc.declare_dram_parameter("b_map", [1], f32, isOutput=False)
    W_gcn = nc.declare_dram_parameter("W_gcn", [DG, DG], f32, isOutput=False)
    out_sh = nc.declare_dram_parameter("out_shard", [S, DG], f32, isOutput=True)

    gat_in = nc.dram_tensor("gat_in", [S * DG], bf16)
    gat_all = nc.dram_tensor("gat_all", [M_CORES * S * DG], bf16, addr_space="Shared")

    with tile.TileContext(nc) as tc:
        with (
            tc.tile_pool(name="stat", bufs=1) as stat,
            tc.tile_pool(name="setup", bufs=2) as setup,
            tc.tile_pool(name="tpool", bufs=NPAIR) as tpool,
            tc.tile_pool(name="ring", bufs=2) as ring,
            tc.tile_pool(name="tail8", bufs=8) as tail8,
            tc.tile_pool(name="ps_acc", bufs=1, space="PSUM") as ps_acc,
            tc.tile_pool(name="psx", bufs=2, space="PSUM") as psx,
        ):
            for _rep in range(reps):
                # ---------------- constants (gpsimd work first) ----------------
                ident_f = stat.tile([P, P], f32, tag="ident_f")
                make_identity(nc, ident_f[:])
                ident_b = stat.tile([P, P], bf16, tag="ident_b")
                make_identity(nc, ident_b[:])
                ones_row = stat.tile([1, P], f32, tag="ones_row")
                nc.gpsimd.memset(ones_row[:], 1.0)
                bigH = stat.tile([P, NCT * (DG + 1)], bf16, tag="bigH")
                nc.gpsimd.memset(
                    bigH[:].rearrange("p (ct w) -> p ct w", w=DG + 1)[:, :, DG : DG + 1],
                    1.0,
                )

                wmap_sb = stat.tile([DIN, DG], f32, tag="wmap")
                nc.sync.dma_start(wmap_sb[:], W_map[:])
                u_sb = stat.tile([DG, 1], f32, tag="u")
                nc.sync.dma_start(u_sb[:], U_in[:])
                v_sb = stat.tile([DG, 1], f32, tag="v")
                nc.sync.dma_start(v_sb[:], V_in[:])
                b_sb = stat.tile([1, 1], f32, tag="b")
                nc.sync.dma_start(b_sb[:], b_in[:])
                wgcn_sb = stat.tile([DG, DG], f32, tag="wgcn")
                nc.sync.dma_start(wgcn_sb[:], W_gcn[:])

                # ---------------- T cast-load stream (starts immediately) ------
                t_tiles = []
                for p in range(NPAIR):
                    tp = tpool.tile([P, F2], bf16, tag="T")
                    t_tiles.append(tp)
                    for half in range(2):
                        nc.gpsimd.dma_start(
                            tp[:, half * S : (half + 1) * S],
                            supT[p * 256 + half * P : p * 256 + (half + 1) * P, :],
                        )

                # ---------------- setup: t first, then s / bigH ----------------
                # t for the shard: hST = W_map^T @ featTs ; t = V^T @ hST + b
                fsh = setup.tile([DIN, S], f32, tag="fch")
                nc.sync.dma_start(fsh[:], featTs[:])
                hst = setup.tile([DG, S], f32, tag="hch")
                for half in range(2):
                    ps_h2 = psx.tile([DG, 512], f32, tag="psr")
                    nc.tensor.matmul(
                        ps_h2[:], wmap_sb[:], fsh[:, half * 512 : (half + 1) * 512],
                        start=True, stop=True,
                    )
                    nc.vector.tensor_copy(hst[:, half * 512 : (half + 1) * 512], ps_h2[:])
                t_row = stat.tile([1, S], f32, tag="t_row")
                for half in range(2):
                    ps_t = psx.tile([1, 512], f32, tag="psr")
                    nc.tensor.matmul(
                        ps_t[:], v_sb[:], hst[:, half * 512 : (half + 1) * 512],
                        start=True, stop=True,
                    )
                    nc.scalar.activation(
                        t_row[:, half * 512 : (half + 1) * 512], ps_t[:],
                        Act.Identity, bias=b_sb[:], scale=1.0,
                    )
                # broadcast t to 128 partitions via ones-matmul (keeps PL free)
                t_bc = stat.tile([P, S], f32, tag="t_bc")
                for half in range(2):
                    ps_b = psx.tile([P, 512], f32, tag="psr")
                    nc.tensor.matmul(
                        ps_b[:], ones_row[:], t_row[:, half * 512 : (half + 1) * 512],
                        start=True, stop=True,
                    )
                    nc.vector.tensor_copy(t_bc[:, half * 512 : (half + 1) * 512], ps_b[:])

                # s and h-tiles, chunk by chunk; s in 8 small tiles so the main
                # loop can start as soon as the first chunk is done.
                s_tiles = []
                for g in range(8):
                    fch = setup.tile([DIN, S], f32, tag="fch")
                    nc.sync.dma_start(fch[:], featT[:, g * S : (g + 1) * S])
                    hch = setup.tile([DG, S], f32, tag="hch")
                    for half in range(2):
                        ps_h = psx.tile([DG, 512], f32, tag="psr")
                        nc.tensor.matmul(
                            ps_h[:],
                            wmap_sb[:],
                            fch[:, half * 512 : (half + 1) * 512],
                            start=True,
                            stop=True,
                        )
                        nc.vector.tensor_copy(
                            hch[:, half * 512 : (half + 1) * 512], ps_h[:]
                        )
                    ps_s = psx.tile([P, 8], f32, tag="psr")
                    for k in range(8):
                        nc.tensor.matmul(
                            ps_s[:, k : k + 1],
                            hch[:, k * P : (k + 1) * P],
                            u_sb[:],
                            start=True,
                            stop=True,
                        )
                    s_g = stat.tile([P, 8], f32, tag=f"s_{g}")
                    s_tiles.append(s_g)
                    nc.vector.tensor_copy(s_g[:], ps_s[:])
                    for k in range(8):
                        ct = g * 8 + k
                        ps_bh = psx.tile([P, DG], f32, tag="psr")
                        nc.tensor.matmul(
                            ps_bh[:],
                            fch[:, k * P : (k + 1) * P],
                            wmap_sb[:],
                            start=True,
                            stop=True,
                        )
                        nc.vector.tensor_copy(
                            bigH[:, ct * (DG + 1) : ct * (DG + 1) + DG], ps_bh[:]
                        )

                # PSUM accumulators (1 bank each)
                un0 = ps_acc.tile([DG + 1, 512], f32, tag="un0")
                un1 = ps_acc.tile([DG + 1, 512], f32, tag="un1")
                unnorm = (un0, un1)
                m0 = ps_acc.tile([DG, 512], f32, tag="m0")
                m1 = ps_acc.tile([DG, 512], f32, tag="m1")
                mm = (m0, m1)

                # ---------------- phase 1 main loop ----------------
                for p in ([] if skip_main else range(NPAIR)):
                    tp = t_tiles[p]
                    z = ring.tile([P, F2], bf16, tag="z")
                    for half in range(2):
                        ct = 2 * p + half
                        nc.scalar.activation(
                            z[:, half * S : (half + 1) * S], t_bc[:],
                            Act.Tanh, bias=s_tiles[ct // 8][:, ct % 8 : ct % 8 + 1],
                            scale=1.0,
                        )
                    e = ring.tile([P, F2], bf16, tag="e")
                    nc.scalar.activation(e[:], z[:], Act.Exp)
                    n = ring.tile([P, F2], bf16, tag="n")
                    if use_ts_tt:
                        msk = setup.tile([P, F2], bf16, tag="hch")
                        nc.vector.tensor_scalar(
                            msk[:], tp[:], 0.0, None, Alu.is_gt
                        )
                        nc.vector.tensor_tensor(
                            n[:], msk[:], e[:], Alu.mult
                        )
                    else:
                        nc.vector.scalar_tensor_tensor(
                            n[:], tp[:], 0.0, e[:], Alu.is_gt, Alu.mult
                        )
                    for half in range(2):
                        ct = 2 * p + half
                        lhs = bigH[:, ct * (DG + 1) : (ct + 1) * (DG + 1)]
                        for jb in range(2):
                            nc.tensor.matmul(
                                unnorm[jb][:],
                                lhs,
                                n[:, half * S + jb * 512 : half * S + (jb + 1) * 512],
                                start=(p == 0 and half == 0),
                                stop=(p == NPAIR - 1 and half == 1),
                            )

                if skip_tail:
                    zz = tail8.tile([P, DG], f32, tag='fin')
                    nc.vector.memset(zz[:], 0.0)
                    for q in range(8):
                        nc.sync.dma_start(out_sh[q * P : (q + 1) * P, :], zz[:])
                else:
                    # ---------------- tail: gat, all-gather -----------------------
                    d_sb = stat.tile([1, S], f32, tag="d_sb")
                    for jb in range(2):
                        nc.scalar.activation(
                            d_sb[:, jb * 512 : (jb + 1) * 512],
                            unnorm[jb][DG : DG + 1, :], Act.Copy,
                        )
                    rec = ring.tile([DG, S], f32, tag="n")
                    for jb in range(2):
                        ps_d = psx.tile([DG, 512], f32, tag="pst")
                        nc.tensor.matmul(
                            ps_d[:], ones_row[:, 0:DG],
                            d_sb[:, jb * 512 : (jb + 1) * 512],
                            start=True, stop=True,
                        )
                        nc.vector.reciprocal(rec[:, jb * 512 : (jb + 1) * 512], ps_d[:])
                    gv = ring.tile([DG, S], f32, tag="z")
                    for jb in range(2):
                        nc.vector.tensor_mul(
                            gv[:, jb * 512 : (jb + 1) * 512],
                            unnorm[jb][0:DG, :],
                            rec[:, jb * 512 : (jb + 1) * 512],
                        )
                    gatT = stat.tile([DG, S], bf16, tag="gatT")
                    nc.scalar.activation(gatT[:], gv[:], Act.Tanh)
                    # transpose gatT -> gat natural [1024, 64]; one batched
                    # store, ONE collective (fixed cost dominates), two
                    # ct-major reloads.
                    gn = tail8.tile([P, 8 * DG], bf16, tag="gn")
                    for q in range(8):
                        ps_g = psx.tile([P, DG], bf16, tag="pst")
                        nc.tensor.transpose(
                            ps_g[:], gatT[:, q * P : (q + 1) * P],
                            ident_b[0:DG, 0:DG],
                        )
                        nc.vector.tensor_copy(
                            gn[:, q * DG : (q + 1) * DG], ps_g[:]
                        )
                    nc.sync.dma_start(
                        gat_in[:].rearrange("(q p d) -> p q d", q=8, p=P),
                        gn[:].rearrange("p (q d) -> p q d", d=DG),
                    )
                    nc.gpsimd.collective_compute(
                        "AllGather",
                        Alu.bypass,
                        replica_groups=[list(range(M_CORES))],
                        ins=[gat_in[:]],
                        outs=[gat_all[:]],
                    )
                    gat_sb = []
                    for hh in range(2):
                        gsb = setup.tile([P, 32 * DG], bf16, tag="fch")
                        gat_sb.append(gsb)
                        nc.sync.dma_start(
                            gsb[:].rearrange("p (ct d) -> p ct d", d=DG),
                            gat_all[
                                hh * 32 * P * DG : (hh + 1) * 32 * P * DG
                            ].rearrange("(ct p d) -> p ct d", p=P, d=DG),
                        )

                    if skip_p2:
                        zz2 = tail8.tile([P, DG], f32, tag='fin')
                        nc.vector.memset(zz2[:], 0.0)
                        for q in range(8):
                            nc.sync.dma_start(out_sh[q * P : (q + 1) * P, :], zz2[:])
                    else:
                        # ---------------- phase 2 ----------------
                        first = True
                        for hh in range(2):
                            for rank in range(8):
                                for k in range(4):
                                    ct = hh * 32 + rank * 4 + k
                                    p_idx, half = ct // 2, ct % 2
                                    lhs = gat_sb[hh][
                                        :, (rank * 4 + k) * DG : (rank * 4 + k + 1) * DG
                                    ]
                                    for jb in range(2):
                                        nc.tensor.matmul(
                                            mm[jb][:],
                                            lhs,
                                            t_tiles[p_idx][
                                                :,
                                                half * S + jb * 512 : half * S
                                                + (jb + 1) * 512,
                                            ],
                                            start=first,
                                            stop=(hh == 1 and rank == 7 and k == 3),
                                        )
                                    first = False
                        m_sb = ring.tile([DG, S], f32, tag="e")
                        for jb in range(2):
                            nc.vector.tensor_copy(m_sb[:, jb * 512 : (jb + 1) * 512], mm[jb][:])
                        reluT = ring.tile([DG, S], f32, tag="n")
                        for jb in range(2):
                            ps_o = psx.tile([DG, 512], f32, tag="pst")
                            nc.tensor.matmul(
                                ps_o[:], wgcn_sb[:], m_sb[:, jb * 512 : (jb + 1) * 512],
                                start=True, stop=True,
                            )
                            nc.scalar.activation(
                                reluT[:, jb * 512 : (jb + 1) * 512], ps_o[:], Act.Relu
                            )

                        # ---------------- normalize + store ----------------
                        # All Squares first (same ACT table as exp/tanh), then one Sqrt
                        # (single table switch), then DVE.
                        onats = []
                        n2_all = stat.tile([P, 8], f32, tag="n2_all")
                        sqs = stat.tile([P, DG], f32, tag="sqs")
                        for q in range(8):
                            ps_t2 = psx.tile([P, DG], f32, tag="pst")
                            nc.tensor.transpose(
                                ps_t2[:], reluT[:, q * P : (q + 1) * P], ident_f[0:DG, 0:DG]
                            )
                            onat = tail8.tile([P, DG], f32, tag="onat")
                            nc.vector.tensor_copy(onat[:], ps_t2[:])
                            onats.append(onat)
                            nc.scalar.activation(
                                sqs[:], ps_t2[:], Act.Square, accum_out=n2_all[:, q : q + 1]
                            )
                        nrm = stat.tile([P, 8], f32, tag="nrm")
                        nc.scalar.activation(nrm[:], n2_all[:], Act.Sqrt)
                        nc.vector.tensor_scalar_max(nrm[:], nrm[:], 1e-12)
                        rcl = stat.tile([P, 8], f32, tag="rcl")
                        nc.vector.reciprocal(rcl[:], nrm[:])
                        for q in range(8):
                            fin = tail8.tile([P, DG], f32, tag="fin")
                            nc.vector.tensor_scalar_mul(fin[:], onats[q][:], rcl[:, q : q + 1])
                            nc.sync.dma_start(out_sh[q * P : (q + 1) * P, :], fin[:])

    if not nc.is_finalized():
        nc.finalize()
    return nc


def _get_nc(reps=1):
    if reps not in _built:
        _built[reps] = _build(reps)
    return _built[reps]


def _make_in_maps(feat, sup, W_map, b_map, U, V, W_gcn):
    feat = np.ascontiguousarray(np.asarray(feat, dtype=np.float32))
    sup = np.asarray(sup, dtype=np.float32)
    W_map_np = np.ascontiguousarray(np.asarray(W_map, dtype=np.float32))
    U_np = np.ascontiguousarray(np.asarray(U, dtype=np.float32))
    V_np = np.ascontiguousarray(np.asarray(V, dtype=np.float32))
    b_np = np.ascontiguousarray(np.asarray(b_map, dtype=np.float32))
    W_gcn_np = np.ascontiguousarray(np.asarray(W_gcn, dtype=np.float32))

    featT = np.ascontiguousarray(feat.T)
    idx = np.arange(S)
    in_maps = []
    for r in range(M_CORES):
        shard = np.array(sup[r * S : (r + 1) * S, :], dtype=np.float32, copy=True)
        shard[idx, r * S + idx] += 1.0  # self loops
        in_maps.append(
            {
                "supT": np.ascontiguousarray(shard.T),
                "featT": featT,
                "featTs": np.ascontiguousarray(featT[:, r * S : (r + 1) * S]),
                "W_map": W_map_np,
                "U": U_np,
                "V": V_np,
                "b_map": b_np,
                "W_gcn": W_gcn_np,
            }
        )
    return in_maps


def kernel(feat, sup, W_map, b_map, U, V, W_gcn):
    from concourse.bass_utils import run_bass_kernel_spmd

    in_maps = _make_in_maps(feat, sup, W_map, b_map, U, V, W_gcn)
    nc = _get_nc()
    trace = bool(int(os.environ.get("KERNEL_TRACE", "0")))
    try:
        res = run_bass_kernel_spmd(
            nc, in_maps, core_ids=list(range(M_CORES)), trace=trace,
            stitch_traces=False,
        )
    except Exception:
        if not trace:
            raise
        res = run_bass_kernel_spmd(
            nc, in_maps, core_ids=list(range(M_CORES)), trace=False,
            stitch_traces=False,
        )
    if trace and res.exec_time_ns is not None:
        print(f"HW exec time: {res.exec_time_ns} ns")
        kernel.last_exec_time_ns = res.exec_time_ns
        kernel.last_results = res
    out = np.concatenate(
        [res.results[r]["out_shard"] for r in range(M_CORES)], axis=0
    )
    return out.astype(np.float32)



# revision 19
# speedup vs baseline: 1.4873x; 1.4873x over previous
"""AttGNN kernel for 8 Trainium2 NeuronCores (Bass/Tile).

Math (reference):
    sup2 = sup + I
    h    = feat @ W_map                      [N, 64]
    s    = h @ U ; t = h @ V                 [N, 1]
    att  = softmax_rows(mask(tanh(s_i + t_j + b), sup2[j, i] > 0))   [N, N]
    gat  = tanh(att @ h)                     [N, 64]
    out  = normalize_rows(relu((sup2 @ gat) @ W_gcn))                [N, 64]

Distribution: 1D row-shard of sup/att over 8 cores (1024 rows each).
Both the attention mask and the two big matmuls need sup2 with the
*global* node index on SBUF partitions, i.e. the transpose of the shard
(T[c, j'] = sup2[shard j', c]).  The per-core input buffer is marshalled
host-side in that layout, pre-cast to bf16 (halves HBM traffic and
keeps the load on HWDGE instead of a gpsimd cast stream).

Softmax trick: softmax is scale-invariant, so exp(tanh(z)) can be
replaced by any g(z) with log g(z) = tanh(z) + const to within the
error budget.  g(z) = sigmoid(A z + B) + D with (A, B, D) fit by
minimax in log space matches within +/-0.32%, turning two full ACT
passes (tanh, exp) over the N x S attention block into one sigmoid
pass.  A is folded into U, V host-side; B (+ A*b_map) rides the
per-partition activation bias; D and the mask are applied on DVE:
    maskC = (T > 0) * 2.0          # tensor_scalar, 4x mode
    g     = sigmoid(t' + s'_c)     # ACT, one pass
    g    += D                      # tensor_scalar in-place, 4x mode
    n     = min(maskC, g)          # tensor_tensor, 2x mode
(min works because 0 < g <= 1+D < 2 everywhere.)

Per core (c = global node index, 64 tiles of 128; j' = local shard row):
  phase 1:  unnorm[65, j'] = sum_c [h | 1][c, :]^T n[c, j']   (PE, PSUM acc)
            row 64 is the softmax denominator d[j'].
            gat[j', :] = tanh(unnorm[0:64, j'] / d[j'])
  all-gather gat (bf16) -> full [8192, 64]
  phase 2:  M[d, i'] = sum_j gat[j, d] T[j, i']               (PE)
            pre[e, i'] = W_gcn^T M                            (PE)
            out[i', :] = normalize(relu(pre))^T               -> store
"""

import os
import numpy as np

N = 8192
DIN = 128
DG = 64
M_CORES = 8
S = N // M_CORES          # 1024 shard rows per core
P = 128                   # partitions
NCT = N // P              # 64 c-tiles
NPAIR = NCT // 2          # 32 pairs of c-tiles
F2 = 2 * S                # 2048 free elems per pair tile

# minimax fit of log(sigmoid(A z + B) + D) ~ tanh(z) + const  (z in [-13, 13])
SIG_A = 2.14235191
SIG_B = -0.99688723
SIG_D = 0.15764918

_built = {}


def _build(reps=1):
    skip_tail = bool(int(os.environ.get("K_SKIP_TAIL", "0")))
    skip_p2 = bool(int(os.environ.get("K_SKIP_P2", "0")))
    skip_main = bool(int(os.environ.get("K_SKIP_MAIN", "0")))
    pool_tt = int(os.environ.get("K_POOL_TT", "8"))
    import concourse.bass as bass
    import concourse.bacc as bacc
    import concourse.mybir as mybir
    import concourse.tile as tile
    from concourse.masks import make_identity

    f32 = mybir.dt.float32
    bf16 = mybir.dt.bfloat16
    Alu = mybir.AluOpType
    Act = mybir.ActivationFunctionType

    nc = bacc.Bacc(None)

    supT = nc.declare_dram_parameter("supT", [N, S], bf16, isOutput=False)
    featT = nc.declare_dram_parameter("featT", [DIN, N], bf16, isOutput=False)
    featTs = nc.declare_dram_parameter("featTs", [DIN, S], bf16, isOutput=False)
    W_map = nc.declare_dram_parameter("W_map", [DIN, DG], bf16, isOutput=False)
    # wu = W_map @ (A U), wv = W_map @ (A V): s' = featT^T wu, t' = featTs^T wv
    wu_in = nc.declare_dram_parameter("wu", [DIN, 1], bf16, isOutput=False)
    wv_in = nc.declare_dram_parameter("wv", [DIN, 1], bf16, isOutput=False)
    bfit_in = nc.declare_dram_parameter("bfit", [1], f32, isOutput=False)
    W_gcn = nc.declare_dram_parameter("W_gcn", [DG, DG], f32, isOutput=False)
    out_sh = nc.declare_dram_parameter("out_shard", [S, DG], f32, isOutput=True)

    gat_in = nc.dram_tensor("gat_in", [S * DG], bf16)
    gat_all = nc.dram_tensor("gat_all", [M_CORES * S * DG], bf16, addr_space="Shared")

    with tile.TileContext(nc) as tc:
        with (
            tc.tile_pool(name="stat", bufs=1) as stat,
            tc.tile_pool(name="setup", bufs=2) as setup,
            tc.tile_pool(name="tpool", bufs=NPAIR) as tpool,
            tc.tile_pool(name="ring", bufs=2) as ring,
            tc.tile_pool(name="tail8", bufs=8) as tail8,
            tc.tile_pool(name="ps_acc", bufs=1, space="PSUM") as ps_acc,
            tc.tile_pool(name="psx", bufs=2, space="PSUM") as psx,
        ):
            for _rep in range(reps):
                # ---------------- constants ----------------
                ident_f = stat.tile([P, P], f32, tag="ident_f")
                make_identity(nc, ident_f[:])
                ident_b = stat.tile([P, P], bf16, tag="ident_b")
                make_identity(nc, ident_b[:])
                ones_row = stat.tile([1, P], f32, tag="ones_row")
                nc.gpsimd.memset(ones_row[:], 1.0)
                bigH = stat.tile([P, NCT * (DG + 1)], bf16, tag="bigH")
                nc.gpsimd.memset(
                    bigH[:].rearrange("p (ct w) -> p ct w", w=DG + 1)[:, :, DG : DG + 1],
                    1.0,
                )

                wmap_sb = stat.tile([DIN, DG], bf16, tag="wmap")
                nc.sync.dma_start(wmap_sb[:], W_map[:])
                wu_sb = stat.tile([DIN, 1], bf16, tag="wu")
                nc.sync.dma_start(wu_sb[:], wu_in[:])
                wv_sb = stat.tile([DIN, 1], bf16, tag="wv")
                nc.sync.dma_start(wv_sb[:], wv_in[:])
                b_sb = stat.tile([1, 1], f32, tag="b")
                nc.sync.dma_start(b_sb[:], bfit_in[:])
                wgcn_sb = stat.tile([DG, DG], f32, tag="wgcn")
                nc.sync.dma_start(wgcn_sb[:], W_gcn[:])
                # broadcast bfit to [P, 1] for per-partition activation bias
                ps_bb = psx.tile([P, 1], f32, tag="psr")
                nc.tensor.matmul(ps_bb[:], ones_row[:], b_sb[:], start=True, stop=True)
                b_bc = stat.tile([P, 1], f32, tag="b_bc")
                nc.vector.tensor_copy(b_bc[:], ps_bb[:])

                # ---------------- setup input loads (ahead of the T stream,
                # they gate t_bc / s / bigH and are small) ------------------
                fsh = setup.tile([DIN, S], bf16, tag="fch")
                nc.sync.dma_start(fsh[:], featTs[:])
                f_chunks = []
                for g in range(8):
                    fch = setup.tile([DIN, S], bf16, tag=f"fg{g}", bufs=1)
                    f_chunks.append(fch)
                    nc.sync.dma_start(fch[:], featT[:, g * S : (g + 1) * S])

                # ---------------- T load stream ----------------------------
                t_tiles = []
                for p in range(NPAIR):
                    tp = tpool.tile([P, F2], bf16, tag="T")
                    t_tiles.append(tp)
                    for half in range(2):
                        nc.sync.dma_start(
                            tp[:, half * S : (half + 1) * S],
                            supT[p * 256 + half * P : p * 256 + (half + 1) * P, :],
                        )

                # ---------------- setup: t first, then s / bigH ----------------
                # t' for the shard: t' = (W_map V')^T featTs = wv^T featTs
                t_row = stat.tile([1, S], f32, tag="t_row")
                for half in range(2):
                    ps_t = psx.tile([1, 512], f32, tag="psr")
                    nc.tensor.matmul(
                        ps_t[:], wv_sb[:], fsh[:, half * 512 : (half + 1) * 512],
                        start=True, stop=True,
                    )
                    nc.vector.tensor_copy(t_row[:, half * 512 : (half + 1) * 512], ps_t[:])
                # broadcast t' to 128 partitions via ones-matmul (keeps PL free)
                t_bc = stat.tile([P, S], bf16, tag="t_bc")
                for half in range(2):
                    ps_b = psx.tile([P, 512], f32, tag="psr")
                    nc.tensor.matmul(
                        ps_b[:], ones_row[:], t_row[:, half * 512 : (half + 1) * 512],
                        start=True, stop=True,
                    )
                    nc.vector.tensor_copy(t_bc[:, half * 512 : (half + 1) * 512], ps_b[:])

                # s' and h-tiles, chunk by chunk; s' in 8 small tiles so the
                # main loop can start as soon as the first chunk is done.
                sb_tiles = []
                for g in range(8):
                    fch = f_chunks[g]
                    # s' column per c-tile: featT_chunk^T @ wu
                    ps_s = psx.tile([P, 8], f32, tag="psr")
                    for k in range(8):
                        nc.tensor.matmul(
                            ps_s[:, k : k + 1],
                            fch[:, k * P : (k + 1) * P],
                            wu_sb[:],
                            start=True,
                            stop=True,
                        )
                    # s_b = s' + (A*b_map + B): per-partition sigmoid bias
                    s_g = stat.tile([P, 8], f32, tag=f"s_{g}")
                    sb_tiles.append(s_g)
                    nc.scalar.activation(
                        s_g[:], ps_s[:], Act.Identity, bias=b_bc[:], scale=1.0
                    )
                    # bigH h-chunks: batch the 8 PSUM->SBUF copies into one
                    ps_bh = psx.tile([P, 8 * DG], f32, tag="psr")
                    for k in range(8):
                        nc.tensor.matmul(
                            ps_bh[:, k * DG : (k + 1) * DG],
                            fch[:, k * P : (k + 1) * P],
                            wmap_sb[:],
                            start=True,
                            stop=True,
                        )
                    nc.vector.tensor_copy(
                        bigH[:, g * 8 * (DG + 1) : (g + 1) * 8 * (DG + 1)]
                        .rearrange("p (ct w) -> p ct w", w=DG + 1)[:, :, 0:DG],
                        ps_bh[:].rearrange("p (ct w) -> p ct w", w=DG),
                    )

                # PSUM accumulators (1 bank each)
                un0 = ps_acc.tile([DG + 1, 512], f32, tag="un0")
                un1 = ps_acc.tile([DG + 1, 512], f32, tag="un1")
                unnorm = (un0, un1)
                m0 = ps_acc.tile([DG, 512], f32, tag="m0")
                m1 = ps_acc.tile([DG, 512], f32, tag="m1")
                mm = (m0, m1)

                # ---------------- phase 1 main loop ----------------
                for p in ([] if skip_main else range(NPAIR)):
                    tp = t_tiles[p]
                    g = ring.tile([P, F2], bf16, tag="g")
                    for half in range(2):
                        ct = 2 * p + half
                        nc.scalar.activation(
                            g[:, half * S : (half + 1) * S], t_bc[:],
                            Act.Sigmoid, bias=sb_tiles[ct // 8][:, ct % 8 : ct % 8 + 1],
                            scale=1.0,
                        )
                    nc.vector.tensor_scalar(g[:], g[:], SIG_D, None, Alu.add)
                    mc = ring.tile([P, F2], bf16, tag="mc")
                    nc.vector.tensor_scalar(mc[:], tp[:], 0.0, 2.0, Alu.is_gt, Alu.mult)
                    n = ring.tile([P, F2], bf16, tag="n")
                    nc.vector.tensor_tensor(n[:], mc[:], g[:], Alu.min)
                    for half in range(2):
                        ct = 2 * p + half
                        lhs = bigH[:, ct * (DG + 1) : (ct + 1) * (DG + 1)]
                        for jb in range(2):
                            nc.tensor.matmul(
                                unnorm[jb][:],
                                lhs,
                                n[:, half * S + jb * 512 : half * S + (jb + 1) * 512],
                                start=(p == 0 and half == 0),
                                stop=(p == NPAIR - 1 and half == 1),
                            )

                if skip_tail:
                    zz = tail8.tile([P, DG], f32, tag='fin')
                    nc.vector.memset(zz[:], 0.0)
                    for q in range(8):
                        nc.sync.dma_start(out_sh[q * P : (q + 1) * P, :], zz[:])
                else:
                    # ---------------- tail: gat, all-gather -----------------------
                    d_sb = stat.tile([1, S], f32, tag="d_sb")
                    for jb in range(2):
                        nc.scalar.activation(
                            d_sb[:, jb * 512 : (jb + 1) * 512],
                            unnorm[jb][DG : DG + 1, :], Act.Copy,
                        )
                    rec = ring.tile([DG, S], f32, tag="n")
                    for jb in range(2):
                        ps_d = psx.tile([DG, 512], f32, tag="pst")
                        nc.tensor.matmul(
                            ps_d[:], ones_row[:, 0:DG],
                            d_sb[:, jb * 512 : (jb + 1) * 512],
                            start=True, stop=True,
                        )
                        nc.vector.reciprocal(rec[:, jb * 512 : (jb + 1) * 512], ps_d[:])
                    gv = ring.tile([DG, S], f32, tag="g")
                    for jb in range(2):
                        nc.vector.tensor_mul(
                            gv[:, jb * 512 : (jb + 1) * 512],
                            unnorm[jb][0:DG, :],
                            rec[:, jb * 512 : (jb + 1) * 512],
                        )
                    gatT = stat.tile([DG, S], bf16, tag="gatT")
                    nc.scalar.activation(gatT[:], gv[:], Act.Tanh)
                    # transpose gatT -> gat natural [1024, 64]; one batched
                    # store, ONE collective (fixed cost dominates), two
                    # ct-major reloads.
                    gn = stat.tile([P, 8 * DG], bf16, tag="gn")
                    for q in range(8):
                        ps_g = psx.tile([P, DG], bf16, tag="pst")
                        nc.tensor.transpose(
                            ps_g[:], gatT[:, q * P : (q + 1) * P],
                            ident_b[0:DG, 0:DG],
                        )
                        nc.vector.tensor_copy(
                            gn[:, q * DG : (q + 1) * DG], ps_g[:]
                        )
                    nc.sync.dma_start(
                        gat_in[:].rearrange("(q p d) -> p q d", q=8, p=P),
                        gn[:].rearrange("p (q d) -> p q d", d=DG),
                    )
                    nc.gpsimd.collective_compute(
                        "AllGather",
                        Alu.bypass,
                        replica_groups=[list(range(M_CORES))],
                        ins=[gat_in[:]],
                        outs=[gat_all[:]],
                    )
                    gat_sb = []
                    for hh in range(2):
                        gsb = setup.tile([P, 32 * DG], bf16, tag="fch")
                        gat_sb.append(gsb)
                        nc.sync.dma_start(
                            gsb[:].rearrange("p (ct d) -> p ct d", d=DG),
                            gat_all[
                                hh * 32 * P * DG : (hh + 1) * 32 * P * DG
                            ].rearrange("(ct p d) -> p ct d", p=P, d=DG),
                        )

                    if skip_p2:
                        zz2 = tail8.tile([P, DG], f32, tag='fin')
                        nc.vector.memset(zz2[:], 0.0)
                        for q in range(8):
                            nc.sync.dma_start(out_sh[q * P : (q + 1) * P, :], zz2[:])
                    else:
                        # ---------------- phase 2 ----------------
                        first = True
                        for hh in range(2):
                            for rank in range(8):
                                for k in range(4):
                                    ct = hh * 32 + rank * 4 + k
                                    p_idx, half = ct // 2, ct % 2
                                    lhs = gat_sb[hh][
                                        :, (rank * 4 + k) * DG : (rank * 4 + k + 1) * DG
                                    ]
                                    for jb in range(2):
                                        nc.tensor.matmul(
                                            mm[jb][:],
                                            lhs,
                                            t_tiles[p_idx][
                                                :,
                                                half * S + jb * 512 : half * S
                                                + (jb + 1) * 512,
                                            ],
                                            start=first,
                                            stop=(hh == 1 and rank == 7 and k == 3),
                                        )
                                    first = False
                        m_sb = ring.tile([DG, S], f32, tag="mc")
                        for jb in range(2):
                            nc.vector.tensor_copy(m_sb[:, jb * 512 : (jb + 1) * 512], mm[jb][:])
                        reluT = ring.tile([DG, S], f32, tag="n")
                        for jb in range(2):
                            ps_o = psx.tile([DG, 512], f32, tag="pst")
                            nc.tensor.matmul(
                                ps_o[:], wgcn_sb[:], m_sb[:, jb * 512 : (jb + 1) * 512],
                                start=True, stop=True,
                            )
                            nc.scalar.activation(
                                reluT[:, jb * 512 : (jb + 1) * 512], ps_o[:], Act.Relu
                            )

                        # ---------------- normalize + store ----------------
                        onats = []
                        n2_all = stat.tile([P, 8], f32, tag="n2_all")
                        sqs = stat.tile([P, DG], f32, tag="sqs")
                        for q in range(8):
                            ps_t2 = psx.tile([P, DG], f32, tag="pst")
                            nc.tensor.transpose(
                                ps_t2[:], reluT[:, q * P : (q + 1) * P], ident_f[0:DG, 0:DG]
                            )
                            onat = tail8.tile([P, DG], f32, tag="onat")
                            nc.vector.tensor_copy(onat[:], ps_t2[:])
                            onats.append(onat)
                            nc.scalar.activation(
                                sqs[:], ps_t2[:], Act.Square, accum_out=n2_all[:, q : q + 1]
                            )
                        nrm = stat.tile([P, 8], f32, tag="nrm")
                        nc.scalar.activation(nrm[:], n2_all[:], Act.Sqrt)
                        nc.vector.tensor_scalar_max(nrm[:], nrm[:], 1e-12)
                        rcl = stat.tile([P, 8], f32, tag="rcl")
                        nc.vector.reciprocal(rcl[:], nrm[:])
                        for q in range(8):
                            fin = tail8.tile([P, DG], f32, tag="fin")
                            nc.vector.tensor_scalar_mul(fin[:], onats[q][:], rcl[:, q : q + 1])
                            nc.sync.dma_start(out_sh[q * P : (q + 1) * P, :], fin[:])

    if not nc.is_finalized():
        nc.finalize()
    return nc


def _get_nc(reps=1):
    if reps not in _built:
        _built[reps] = _build(reps)
    return _built[reps]


def _make_in_maps(feat, sup, W_map, b_map, U, V, W_gcn):
    import ml_dtypes

    bf = ml_dtypes.bfloat16
    feat = np.ascontiguousarray(np.asarray(feat, dtype=np.float32))
    sup = np.asarray(sup, dtype=np.float32)
    W_map_f = np.asarray(W_map, dtype=np.float32)
    W_map_np = np.ascontiguousarray(W_map_f).astype(bf)
    wu_np = np.ascontiguousarray(
        W_map_f @ (SIG_A * np.asarray(U, dtype=np.float32))
    ).astype(bf)
    wv_np = np.ascontiguousarray(
        W_map_f @ (SIG_A * np.asarray(V, dtype=np.float32))
    ).astype(bf)
    b_np = np.asarray(
        SIG_A * np.asarray(b_map, dtype=np.float32).reshape(1) + SIG_B,
        dtype=np.float32,
    )
    W_gcn_np = np.ascontiguousarray(np.asarray(W_gcn, dtype=np.float32))

    featT = np.ascontiguousarray(feat.T).astype(bf)
    idx = np.arange(S)
    in_maps = []
    for r in range(M_CORES):
        shard = np.array(sup[r * S : (r + 1) * S, :], dtype=np.float32, copy=True)
        shard[idx, r * S + idx] += 1.0  # self loops
        in_maps.append(
            {
                "supT": np.ascontiguousarray(shard.T).astype(bf),
                "featT": featT,
                "featTs": np.ascontiguousarray(featT[:, r * S : (r + 1) * S]),
                "W_map": W_map_np,
                "wu": wu_np,
                "wv": wv_np,
                "bfit": b_np,
                "W_gcn": W_gcn_np,
            }
        )
    return in_maps


def kernel(feat, sup, W_map, b_map, U, V, W_gcn):
    from concourse.bass_utils import run_bass_kernel_spmd

    in_maps = _make_in_maps(feat, sup, W_map, b_map, U, V, W_gcn)
    nc = _get_nc()
    trace = bool(int(os.environ.get("KERNEL_TRACE", "0")))
    try:
        res = run_bass_kernel_spmd(
            nc, in_maps, core_ids=list(range(M_CORES)), trace=trace,
            stitch_traces=False,
        )
    except Exception:
        if not trace:
            raise
        res = run_bass_kernel_spmd(
            nc, in_maps, core_ids=list(range(M_CORES)), trace=False,
            stitch_traces=False,
        )
    if trace and res.exec_time_ns is not None:
        print(f"HW exec time: {res.exec_time_ns} ns")
        kernel.last_exec_time_ns = res.exec_time_ns
        kernel.last_results = res
    out = np.concatenate(
        [res.results[r]["out_shard"] for r in range(M_CORES)], axis=0
    )
    return out.astype(np.float32)


# revision 34
# speedup vs baseline: 1.8213x; 1.2246x over previous
"""AttGNN kernel for 8 Trainium2 NeuronCores (Bass/Tile).

Math (reference):
    sup2 = sup + I
    h    = feat @ W_map                      [N, 64]
    s    = h @ U ; t = h @ V                 [N, 1]
    att  = softmax_rows(mask(tanh(s_i + t_j + b), sup2[j, i] > 0))   [N, N]
    gat  = tanh(att @ h)                     [N, 64]
    out  = normalize_rows(relu((sup2 @ gat) @ W_gcn))                [N, 64]

Distribution: 1D row-shard of sup/att over 8 cores (1024 rows each).
Both the attention mask and the two big matmuls need sup2 with the
*global* node index on SBUF partitions, i.e. the transpose of the shard
(T[c, j'] = sup2[shard j', c]).  The per-core input buffer is marshalled
host-side in that layout, pre-cast to bf16 (halves HBM traffic and
keeps the load on HWDGE instead of a gpsimd cast stream).

Softmax trick: softmax is scale-invariant, so exp(tanh(z)) can be
replaced by any g(z) with log g(z) = tanh(z) + const to within the
error budget.  g(z) = sigmoid(A z + B) + D with (A, B, D) fit by
minimax in log space matches within +/-0.32%, turning two full ACT
passes (tanh, exp) over the N x S attention block into one sigmoid
pass.  A is folded into U, V host-side; B (+ A*b_map) rides the
per-partition activation bias; D and the mask are applied on DVE:
    maskC = (T > 0) * 2.0          # tensor_scalar, 4x mode
    g     = sigmoid(t' + s'_c)     # ACT, one pass
    g    += D                      # tensor_scalar in-place, 4x mode
    n     = min(maskC, g)          # tensor_tensor, 2x mode
(min works because 0 < g <= 1+D < 2 everywhere.)

Per core (c = global node index, 64 tiles of 128; j' = local shard row):
  phase 1:  unnorm[65, j'] = sum_c [h | 1][c, :]^T n[c, j']   (PE, PSUM acc)
            row 64 is the softmax denominator d[j'].
            gat[j', :] = tanh(unnorm[0:64, j'] / d[j'])
  all-gather gat (bf16) -> full [8192, 64]
  phase 2:  M[d, i'] = sum_j gat[j, d] T[j, i']               (PE)
            pre[e, i'] = W_gcn^T M                            (PE)
            out[i', :] = normalize(relu(pre))^T               -> store
"""

import os
import numpy as np

N = 8192
DIN = 128
DG = 64
M_CORES = 8
S = N // M_CORES          # 1024 shard rows per core
P = 128                   # partitions
NCT = N // P              # 64 c-tiles
NPAIR = NCT // 2          # 32 pairs of c-tiles
F2 = 2 * S                # 2048 free elems per pair tile

# minimax fit of log(sigmoid(A z + B) + D) ~ tanh(z) + const  (z in [-13, 13])
SIG_A = 2.14235191
SIG_B = -0.99688723
SIG_D = 0.15764918

_built = {}


def _build(reps=1):
    skip_tail = bool(int(os.environ.get("K_SKIP_TAIL", "0")))
    skip_p2 = bool(int(os.environ.get("K_SKIP_P2", "0")))
    skip_main = bool(int(os.environ.get("K_SKIP_MAIN", "0")))
    pool_tt = int(os.environ.get("K_POOL_TT", "8"))
    import concourse.bass as bass
    import concourse.bacc as bacc
    import concourse.mybir as mybir
    import concourse.tile as tile
    from concourse.masks import make_identity

    f32 = mybir.dt.float32
    bf16 = mybir.dt.bfloat16
    fp8 = mybir.dt.float8e4
    Alu = mybir.AluOpType
    Act = mybir.ActivationFunctionType

    nc = bacc.Bacc(None)

    supT = nc.declare_dram_parameter("supT", [N, S], bf16, isOutput=False)
    featT = nc.declare_dram_parameter("featT", [DIN, N], bf16, isOutput=False)
    featTs = nc.declare_dram_parameter("featTs", [DIN, S], bf16, isOutput=False)
    W_map = nc.declare_dram_parameter("W_map", [DIN, DG], bf16, isOutput=False)
    # wu = W_map @ (A U), wv = W_map @ (A V): s' = featT^T wu, t' = featTs^T wv
    wu_in = nc.declare_dram_parameter("wu", [DIN, 1], bf16, isOutput=False)
    wv_in = nc.declare_dram_parameter("wv", [DIN, 1], bf16, isOutput=False)
    bfit_in = nc.declare_dram_parameter("bfit", [1], f32, isOutput=False)
    W_gcn = nc.declare_dram_parameter("W_gcn", [DG, DG], f32, isOutput=False)
    out_sh = nc.declare_dram_parameter("out_shard", [S, DG], f32, isOutput=True)

    gat_in = nc.dram_tensor("gat_in", [S * DG], fp8)
    gat_all = nc.dram_tensor("gat_all", [M_CORES * S * DG], fp8, addr_space="Shared")

    with tile.TileContext(nc) as tc:
        with (
            tc.tile_pool(name="stat", bufs=1) as stat,
            tc.tile_pool(name="setup", bufs=2) as setup,
            tc.tile_pool(name="tpool", bufs=NPAIR) as tpool,
            tc.tile_pool(name="ring", bufs=2) as ring,
            tc.tile_pool(name="tail8", bufs=8) as tail8,
            tc.tile_pool(name="ps_acc", bufs=1, space="PSUM") as ps_acc,
            tc.tile_pool(name="psx", bufs=2, space="PSUM") as psx,
        ):
            for _rep in range(reps):
                # ---------------- constants ----------------
                ident_f = stat.tile([P, P], f32, tag="ident_f")
                make_identity(nc, ident_f[:])
                ident_b = stat.tile([P, P], bf16, tag="ident_b")
                make_identity(nc, ident_b[:])
                ones_row = stat.tile([1, P], f32, tag="ones_row")
                nc.gpsimd.memset(ones_row[:], 1.0)
                bigH = stat.tile([P, NCT * (DG + 1)], bf16, tag="bigH")
                nc.gpsimd.memset(
                    bigH[:].rearrange("p (ct w) -> p ct w", w=DG + 1)[:, :, DG : DG + 1],
                    1.0,
                )


                wmap_sb = stat.tile([DIN, DG], bf16, tag="wmap")
                nc.sync.dma_start(wmap_sb[:], W_map[:])
                wu_sb = stat.tile([DIN, 1], bf16, tag="wu")
                nc.sync.dma_start(wu_sb[:], wu_in[:])
                wv_sb = stat.tile([DIN, 1], bf16, tag="wv")
                nc.sync.dma_start(wv_sb[:], wv_in[:])
                b_sb = stat.tile([1, 1], f32, tag="b")
                nc.sync.dma_start(b_sb[:], bfit_in[:])
                wgcn_sb = stat.tile([DG, DG], f32, tag="wgcn")
                nc.sync.dma_start(wgcn_sb[:], W_gcn[:])
                # broadcast bfit to [P, 1] for per-partition activation bias
                ps_bb = psx.tile([P, 1], f32, tag="ps")
                nc.tensor.matmul(ps_bb[:], ones_row[:], b_sb[:], start=True, stop=True)
                b_bc = stat.tile([P, 1], f32, tag="b_bc")
                nc.vector.tensor_copy(b_bc[:], ps_bb[:])

                # ---------------- setup input loads (ahead of the T stream,
                # they gate t_bc / s / bigH and are small) ------------------
                fsh = setup.tile([DIN, S], bf16, tag="fch")
                nc.sync.dma_start(fsh[:], featTs[:])
                f_chunks = []
                for g in range(8):
                    fch = setup.tile([DIN, S], bf16, tag=f"fg{g}", bufs=1)
                    f_chunks.append(fch)
                    nc.sync.dma_start(fch[:], featT[:, g * S : (g + 1) * S])

                # ---------------- T load stream ----------------------------
                t_tiles = []
                for p in range(NPAIR):
                    tp = tpool.tile([P, F2], bf16, tag="T")
                    t_tiles.append(tp)
                    for half in range(2):
                        nc.sync.dma_start(
                            tp[:, half * S : (half + 1) * S],
                            supT[p * 256 + half * P : p * 256 + (half + 1) * P, :],
                        )

                # ---------------- setup: t first, then s / bigH ----------------
                # t' for the shard: t' = (W_map V')^T featTs = wv^T featTs
                t_row = stat.tile([1, S], f32, tag="t_row")
                for half in range(2):
                    ps_t = psx.tile([1, 512], f32, tag="ps")
                    nc.tensor.matmul(
                        ps_t[:], wv_sb[:], fsh[:, half * 512 : (half + 1) * 512],
                        start=True, stop=True,
                    )
                    nc.vector.tensor_copy(t_row[:, half * 512 : (half + 1) * 512], ps_t[:])
                # broadcast t' to 128 partitions via ones-matmul (keeps PL free)
                t_bc = stat.tile([P, S], bf16, tag="t_bc")
                for half in range(2):
                    ps_b = psx.tile([P, 512], f32, tag="ps")
                    nc.tensor.matmul(
                        ps_b[:], ones_row[:], t_row[:, half * 512 : (half + 1) * 512],
                        start=True, stop=True,
                    )
                    nc.vector.tensor_copy(t_bc[:, half * 512 : (half + 1) * 512], ps_b[:])

                # s' and h-tiles, chunk by chunk; s' in 8 small tiles so the
                # main loop can start as soon as the first chunk is done.
                sb_tiles = []
                for g in range(8):
                    fch = f_chunks[g]
                    # s' column per c-tile: featT_chunk^T @ wu
                    ps_s = psx.tile([P, 8], f32, tag="ps")
                    for k in range(8):
                        nc.tensor.matmul(
                            ps_s[:, k : k + 1],
                            fch[:, k * P : (k + 1) * P],
                            wu_sb[:],
                            start=True,
                            stop=True,
                        )
                    # s_b = s' + (A*b_map + B): per-partition sigmoid bias
                    s_g = stat.tile([P, 8], f32, tag=f"s_{g}")
                    sb_tiles.append(s_g)
                    nc.scalar.activation(
                        s_g[:], ps_s[:], Act.Identity, bias=b_bc[:], scale=1.0
                    )
                    # bigH h-chunks: batch the 8 PSUM->SBUF copies into one
                    ps_bh = psx.tile([P, 8 * DG], f32, tag="ps")
                    for k in range(8):
                        nc.tensor.matmul(
                            ps_bh[:, k * DG : (k + 1) * DG],
                            fch[:, k * P : (k + 1) * P],
                            wmap_sb[:],
                            start=True,
                            stop=True,
                        )
                    nc.vector.tensor_copy(
                        bigH[:, g * 8 * (DG + 1) : (g + 1) * 8 * (DG + 1)]
                        .rearrange("p (ct w) -> p ct w", w=DG + 1)[:, :, 0:DG],
                        ps_bh[:].rearrange("p (ct w) -> p ct w", w=DG),
                    )


                # PSUM accumulators (1 bank each)
                un0 = ps_acc.tile([DG + 1, 512], f32, tag="un0")
                un1 = ps_acc.tile([DG + 1, 512], f32, tag="un1")
                unnorm = (un0, un1)
                uc0 = ps_acc.tile([DG + 1, 512], f32, tag="uc0")
                uc1 = ps_acc.tile([DG + 1, 512], f32, tag="uc1")
                ucorr = (uc0, uc1)
                m0 = ps_acc.tile([DG, 512], f32, tag="m0")
                m1 = ps_acc.tile([DG, 512], f32, tag="m1")
                mm = (m0, m1)

                # ---------------- phase 1 main loop ----------------
                for p in ([] if skip_main else range(NPAIR)):
                    tp = t_tiles[p]
                    g = ring.tile([P, F2], bf16, tag="g")
                    for half in range(2):
                        ct = 2 * p + half
                        nc.scalar.activation(
                            g[:, half * S : (half + 1) * S], t_bc[:],
                            Act.Sigmoid, bias=sb_tiles[ct // 8][:, ct % 8 : ct % 8 + 1],
                            scale=1.0,
                        )
                    mc = ring.tile([P, F2], bf16, tag="mc")
                    nc.vector.tensor_scalar(mc[:], tp[:], 0.0, 2.0, Alu.is_gt, Alu.mult)
                    n = ring.tile([P, F2], bf16, tag="n")
                    nc.vector.tensor_tensor(n[:], mc[:], g[:], Alu.min)
                    for half in range(2):
                        ct = 2 * p + half
                        lhs = bigH[:, ct * (DG + 1) : (ct + 1) * (DG + 1)]
                        for jb in range(2):
                            sl = slice(half * S + jb * 512, half * S + (jb + 1) * 512)
                            nc.tensor.matmul(
                                unnorm[jb][:], lhs, n[:, sl],
                                start=(p == 0 and half == 0),
                                stop=(p == NPAIR - 1 and half == 1),
                            )
                            # same stationary, moving = mask*2: accumulates
                            # 2*sum(mask*[h|1]); combined as +D/2 in the tail
                            nc.tensor.matmul(
                                ucorr[jb][:], lhs, mc[:, sl],
                                start=(p == 0 and half == 0),
                                stop=(p == NPAIR - 1 and half == 1),
                            )

                if skip_tail:
                    zz = tail8.tile([P, DG], f32, tag='fin')
                    nc.vector.memset(zz[:], 0.0)
                    for q in range(8):
                        nc.sync.dma_start(out_sh[q * P : (q + 1) * P, :], zz[:])
                else:
                    # ---------------- tail: gat, all-gather -----------------------
                    # comb = unnorm + (D/2) * ucorr  (the +D offset term);
                    # stt may read only one PSUM operand, so stage ucorr in SBUF
                    ucsb = ring.tile([DG + 1, S], f32, tag="n")
                    for jb in range(2):
                        nc.vector.tensor_copy(
                            ucsb[:, jb * 512 : (jb + 1) * 512], ucorr[jb][:]
                        )
                    comb = ring.tile([DG + 1, S], f32, tag="g")
                    for jb in range(2):
                        nc.vector.scalar_tensor_tensor(
                            comb[:, jb * 512 : (jb + 1) * 512],
                            ucsb[:, jb * 512 : (jb + 1) * 512], SIG_D / 2.0,
                            unnorm[jb][:],
                            Alu.mult, Alu.add,
                        )
                    d_sb = stat.tile([1, S], f32, tag="d_sb")
                    nc.scalar.activation(d_sb[:], comb[DG : DG + 1, :], Act.Copy)
                    rec = ring.tile([DG, S], f32, tag="n")
                    for jb in range(2):
                        ps_d = psx.tile([DG, 512], f32, tag="ps")
                        nc.tensor.matmul(
                            ps_d[:], ones_row[:, 0:DG],
                            d_sb[:, jb * 512 : (jb + 1) * 512],
                            start=True, stop=True,
                        )
                        nc.vector.reciprocal(rec[:, jb * 512 : (jb + 1) * 512], ps_d[:])
                    gv = ring.tile([DG, S], f32, tag="mc")
                    nc.vector.tensor_mul(gv[:], comb[0:DG, :], rec[:])
                    gatT = stat.tile([DG, S], bf16, tag="gatT")
                    nc.scalar.activation(gatT[:], gv[:], Act.Tanh)
                    # transpose gatT -> gat natural [1024, 64]; one batched
                    # store, ONE collective (fixed cost dominates), two
                    # ct-major reloads.
                    gn = stat.tile([P, 8 * DG], fp8, tag="gn")
                    for q in range(8):
                        ps_g = psx.tile([P, DG], bf16, tag="ps")
                        nc.tensor.transpose(
                            ps_g[:], gatT[:, q * P : (q + 1) * P],
                            ident_b[0:DG, 0:DG],
                        )
                        nc.vector.tensor_copy(
                            gn[:, q * DG : (q + 1) * DG], ps_g[:]
                        )
                    nc.sync.dma_start(
                        gat_in[:].rearrange("(q p d) -> p q d", q=8, p=P),
                        gn[:].rearrange("p (q d) -> p q d", d=DG),
                    )
                    nc.gpsimd.collective_compute(
                        "AllGather",
                        Alu.bypass,
                        replica_groups=[list(range(M_CORES))],
                        ins=[gat_in[:]],
                        outs=[gat_all[:]],
                    )
                    gat_sb = []
                    for hh in range(2):
                        # reuse a dead featT-chunk buffer (same byte size)
                        gsb8 = setup.tile([P, 32 * DG], fp8, tag=f"fg{hh}", bufs=1)
                        nc.sync.dma_start(
                            gsb8[:].rearrange("p (ct d) -> p ct d", d=DG),
                            gat_all[
                                hh * 32 * P * DG : (hh + 1) * 32 * P * DG
                            ].rearrange("(ct p d) -> p ct d", p=P, d=DG),
                        )
                        gsb = setup.tile([P, 32 * DG], bf16, tag="fch")
                        gat_sb.append(gsb)
                        nc.vector.tensor_copy(gsb[:], gsb8[:])

                    if skip_p2:
                        zz2 = tail8.tile([P, DG], f32, tag='fin')
                        nc.vector.memset(zz2[:], 0.0)
                        for q in range(8):
                            nc.sync.dma_start(out_sh[q * P : (q + 1) * P, :], zz2[:])
                    else:
                        # ---------------- phase 2 ----------------
                        first = True
                        for hh in range(2):
                            for rank in range(8):
                                for k in range(4):
                                    ct = hh * 32 + rank * 4 + k
                                    p_idx, half = ct // 2, ct % 2
                                    lhs = gat_sb[hh][
                                        :, (rank * 4 + k) * DG : (rank * 4 + k + 1) * DG
                                    ]
                                    for jb in range(2):
                                        nc.tensor.matmul(
                                            mm[jb][:],
                                            lhs,
                                            t_tiles[p_idx][
                                                :,
                                                half * S + jb * 512 : half * S
                                                + (jb + 1) * 512,
                                            ],
                                            start=first,
                                            stop=(hh == 1 and rank == 7 and k == 3),
                                        )
                                    first = False
                        m_sb = ring.tile([DG, S], f32, tag="mc")
                        for jb in range(2):
                            nc.vector.tensor_copy(m_sb[:, jb * 512 : (jb + 1) * 512], mm[jb][:])
                        reluT = ring.tile([DG, S], f32, tag="n")
                        for jb in range(2):
                            ps_o = psx.tile([DG, 512], f32, tag="ps")
                            nc.tensor.matmul(
                                ps_o[:], wgcn_sb[:], m_sb[:, jb * 512 : (jb + 1) * 512],
                                start=True, stop=True,
                            )
                            nc.scalar.activation(
                                reluT[:, jb * 512 : (jb + 1) * 512], ps_o[:], Act.Relu
                            )

                        # ---------------- normalize + store ----------------
                        onats = []
                        n2_all = stat.tile([P, 8], f32, tag="n2_all")
                        sqs = stat.tile([P, DG], f32, tag="sqs")
                        for q in range(8):
                            ps_t2 = psx.tile([P, DG], f32, tag="ps")
                            nc.tensor.transpose(
                                ps_t2[:], reluT[:, q * P : (q + 1) * P], ident_f[0:DG, 0:DG]
                            )
                            onat = tail8.tile([P, DG], f32, tag="onat")
                            nc.vector.tensor_copy(onat[:], ps_t2[:])
                            onats.append(onat)
                            nc.scalar.activation(
                                sqs[:], ps_t2[:], Act.Square, accum_out=n2_all[:, q : q + 1]
                            )
                        nrm = stat.tile([P, 8], f32, tag="nrm")
                        nc.scalar.activation(nrm[:], n2_all[:], Act.Sqrt)
                        nc.vector.tensor_scalar_max(nrm[:], nrm[:], 1e-12)
                        rcl = stat.tile([P, 8], f32, tag="rcl")
                        nc.vector.reciprocal(rcl[:], nrm[:])
                        for q in range(8):
                            fin = tail8.tile([P, DG], f32, tag="fin")
                            nc.vector.tensor_scalar_mul(fin[:], onats[q][:], rcl[:, q : q + 1])
                            nc.sync.dma_start(out_sh[q * P : (q + 1) * P, :], fin[:])

    if not nc.is_finalized():
        nc.finalize()
    return nc


def _get_nc(reps=1):
    if reps not in _built:
        _built[reps] = _build(reps)
    return _built[reps]


def _make_in_maps(feat, sup, W_map, b_map, U, V, W_gcn):
    import ml_dtypes

    bf = ml_dtypes.bfloat16
    feat = np.ascontiguousarray(np.asarray(feat, dtype=np.float32))
    sup = np.asarray(sup, dtype=np.float32)
    W_map_f = np.asarray(W_map, dtype=np.float32)
    W_map_np = np.ascontiguousarray(W_map_f).astype(bf)
    wu_np = np.ascontiguousarray(
        W_map_f @ (SIG_A * np.asarray(U, dtype=np.float32))
    ).astype(bf)
    wv_np = np.ascontiguousarray(
        W_map_f @ (SIG_A * np.asarray(V, dtype=np.float32))
    ).astype(bf)
    b_np = np.asarray(
        SIG_A * np.asarray(b_map, dtype=np.float32).reshape(1) + SIG_B,
        dtype=np.float32,
    )
    W_gcn_np = np.ascontiguousarray(np.asarray(W_gcn, dtype=np.float32))

    featT = np.ascontiguousarray(feat.T).astype(bf)
    idx = np.arange(S)
    in_maps = []
    for r in range(M_CORES):
        shard = np.array(sup[r * S : (r + 1) * S, :], dtype=np.float32, copy=True)
        shard[idx, r * S + idx] += 1.0  # self loops
        in_maps.append(
            {
                "supT": np.ascontiguousarray(shard.T).astype(bf),
                "featT": featT,
                "featTs": np.ascontiguousarray(featT[:, r * S : (r + 1) * S]),
                "W_map": W_map_np,
                "wu": wu_np,
                "wv": wv_np,
                "bfit": b_np,
                "W_gcn": W_gcn_np,
            }
        )
    return in_maps


def kernel(feat, sup, W_map, b_map, U, V, W_gcn):
    from concourse.bass_utils import run_bass_kernel_spmd

    in_maps = _make_in_maps(feat, sup, W_map, b_map, U, V, W_gcn)
    nc = _get_nc()
    trace = bool(int(os.environ.get("KERNEL_TRACE", "0")))
    try:
        res = run_bass_kernel_spmd(
            nc, in_maps, core_ids=list(range(M_CORES)), trace=trace,
            stitch_traces=False,
        )
    except Exception:
        if not trace:
            raise
        res = run_bass_kernel_spmd(
            nc, in_maps, core_ids=list(range(M_CORES)), trace=False,
            stitch_traces=False,
        )
    if trace and res.exec_time_ns is not None:
        print(f"HW exec time: {res.exec_time_ns} ns")
        kernel.last_exec_time_ns = res.exec_time_ns
        kernel.last_results = res
    out = np.concatenate(
        [res.results[r]["out_shard"] for r in range(M_CORES)], axis=0
    )
    return out.astype(np.float32)


# revision 36
# speedup vs baseline: 2.4574x; 1.3492x over previous
"""AttGNN kernel for 8 Trainium2 NeuronCores (Bass/Tile).

Math (reference):
    sup2 = sup + I
    h    = feat @ W_map                      [N, 64]
    s    = h @ U ; t = h @ V                 [N, 1]
    att  = softmax_rows(mask(tanh(s_i + t_j + b), sup2[j, i] > 0))   [N, N]
    gat  = tanh(att @ h)                     [N, 64]
    out  = normalize_rows(relu((sup2 @ gat) @ W_gcn))                [N, 64]

Distribution: 1D row-shard of sup/att over 8 cores (1024 rows each).
Both the attention mask and the two big matmuls need sup2 with the
*global* node index on SBUF partitions, i.e. the transpose of the shard
(T[c, j'] = sup2[shard j', c]).  The per-core input buffer is marshalled
host-side in that layout, pre-cast to bf16 (halves HBM traffic and
keeps the load on HWDGE instead of a gpsimd cast stream).

Softmax trick: softmax is scale-invariant, so exp(tanh(z)) can be
replaced by any g(z) with log g(z) = tanh(z) + const to within the
error budget.  g(z) = sigmoid(A z + B) + D with (A, B, D) fit by
minimax in log space matches within +/-0.32%, turning two full ACT
passes (tanh, exp) over the N x S attention block into one sigmoid
pass.  A is folded into U, V host-side; B (+ A*b_map) rides the
per-partition activation bias; D and the mask are applied on DVE:
    maskC = (T > 0) * 2.0          # tensor_scalar, 4x mode
    g     = sigmoid(t' + s'_c)     # ACT, one pass
    g    += D                      # tensor_scalar in-place, 4x mode
    n     = min(maskC, g)          # tensor_tensor, 2x mode
(min works because 0 < g <= 1+D < 2 everywhere.)

Per core (c = global node index, 64 tiles of 128; j' = local shard row):
  phase 1:  unnorm[65, j'] = sum_c [h | 1][c, :]^T n[c, j']   (PE, PSUM acc)
            row 64 is the softmax denominator d[j'].
            gat[j', :] = tanh(unnorm[0:64, j'] / d[j'])
  all-gather gat (bf16) -> full [8192, 64]
  phase 2:  M[d, i'] = sum_j gat[j, d] T[j, i']               (PE)
            pre[e, i'] = W_gcn^T M                            (PE)
            out[i', :] = normalize(relu(pre))^T               -> store
"""

import os
import numpy as np

N = 8192
DIN = 128
DG = 64
M_CORES = 8
S = N // M_CORES          # 1024 shard rows per core
P = 128                   # partitions
NCT = N // P              # 64 c-tiles
NPAIR = NCT // 2          # 32 pairs of c-tiles
F2 = 2 * S                # 2048 free elems per pair tile

# minimax fit of log(sigmoid(A z + B) + D) ~ tanh(z) + const  (z in [-13, 13])
SIG_A = 2.14235191
SIG_B = -0.99688723
SIG_D = 0.15764918

_built = {}


def _build(reps=1):
    skip_tail = bool(int(os.environ.get("K_SKIP_TAIL", "0")))
    skip_p2 = bool(int(os.environ.get("K_SKIP_P2", "0")))
    skip_main = bool(int(os.environ.get("K_SKIP_MAIN", "0")))
    pool_tt = int(os.environ.get("K_POOL_TT", "8"))
    import concourse.bass as bass
    import concourse.bacc as bacc
    import concourse.mybir as mybir
    import concourse.tile as tile
    from concourse.masks import make_identity

    f32 = mybir.dt.float32
    bf16 = mybir.dt.bfloat16
    fp8 = mybir.dt.float8e4
    Alu = mybir.AluOpType
    Act = mybir.ActivationFunctionType

    nc = bacc.Bacc(None)

    supT = nc.declare_dram_parameter("supT", [N, S], bf16, isOutput=False)
    featT = nc.declare_dram_parameter("featT", [DIN, N], bf16, isOutput=False)
    featTs = nc.declare_dram_parameter("featTs", [DIN, S], bf16, isOutput=False)
    W_map = nc.declare_dram_parameter("W_map", [DIN, DG], bf16, isOutput=False)
    # wu = W_map @ (A U), wv = W_map @ (A V): s' = featT^T wu, t' = featTs^T wv
    wu_in = nc.declare_dram_parameter("wu", [DIN, 1], bf16, isOutput=False)
    wv_in = nc.declare_dram_parameter("wv", [DIN, 1], bf16, isOutput=False)
    bfit_in = nc.declare_dram_parameter("bfit", [1], f32, isOutput=False)
    W_gcn = nc.declare_dram_parameter("W_gcn", [DG, DG], f32, isOutput=False)
    out_sh = nc.declare_dram_parameter("out_shard", [S, DG], f32, isOutput=True)

    gat_in = nc.dram_tensor("gat_in", [S * DG], fp8)
    gat_all = nc.dram_tensor("gat_all", [M_CORES * S * DG], fp8, addr_space="Shared")

    with tile.TileContext(nc) as tc:
        with (
            tc.tile_pool(name="stat", bufs=1) as stat,
            tc.tile_pool(name="setup", bufs=2) as setup,
            tc.tile_pool(name="tpool", bufs=NPAIR) as tpool,
            tc.tile_pool(name="ring", bufs=2) as ring,
            tc.tile_pool(name="tail8", bufs=8) as tail8,
            tc.tile_pool(name="ps_acc", bufs=1, space="PSUM") as ps_acc,
            tc.tile_pool(name="psx", bufs=2, space="PSUM") as psx,
        ):
            for _rep in range(reps):
                # ---------------- constants ----------------
                ident_f = stat.tile([P, P], f32, tag="ident_f")
                make_identity(nc, ident_f[:])
                ident_b = stat.tile([P, P], bf16, tag="ident_b")
                make_identity(nc, ident_b[:])
                ones_row = stat.tile([1, P], f32, tag="ones_row")
                nc.gpsimd.memset(ones_row[:], 1.0)
                bigH = stat.tile([P, NCT * (DG + 1)], bf16, tag="bigH")
                nc.gpsimd.memset(
                    bigH[:].rearrange("p (ct w) -> p ct w", w=DG + 1)[:, :, DG : DG + 1],
                    1.0,
                )


                wmap_sb = stat.tile([DIN, DG], bf16, tag="wmap")
                nc.sync.dma_start(wmap_sb[:], W_map[:])
                wu_sb = stat.tile([DIN, 1], bf16, tag="wu")
                nc.sync.dma_start(wu_sb[:], wu_in[:])
                wv_sb = stat.tile([DIN, 1], bf16, tag="wv")
                nc.sync.dma_start(wv_sb[:], wv_in[:])
                b_sb = stat.tile([1, 1], f32, tag="b")
                nc.sync.dma_start(b_sb[:], bfit_in[:])
                wgcn_sb = stat.tile([DG, DG], f32, tag="wgcn")
                nc.sync.dma_start(wgcn_sb[:], W_gcn[:])
                # broadcast bfit to [P, 1] for per-partition activation bias
                ps_bb = psx.tile([P, 1], f32, tag="ps")
                nc.tensor.matmul(ps_bb[:], ones_row[:], b_sb[:], start=True, stop=True)
                b_bc = stat.tile([P, 1], f32, tag="b_bc")
                nc.vector.tensor_copy(b_bc[:], ps_bb[:])

                # ---------------- setup input loads (ahead of the T stream,
                # they gate t_bc / s / bigH and are small) ------------------
                fsh = setup.tile([DIN, S], bf16, tag="fch")
                nc.sync.dma_start(fsh[:], featTs[:])
                f_chunks = []
                for g in range(8):
                    fch = setup.tile([DIN, S], bf16, tag=f"fg{g}", bufs=1)
                    f_chunks.append(fch)
                    nc.sync.dma_start(fch[:], featT[:, g * S : (g + 1) * S])

                # ---------------- T load stream ----------------------------
                t_tiles = []
                for p in range(NPAIR):
                    tp = tpool.tile([P, F2], bf16, tag="T")
                    t_tiles.append(tp)
                    for half in range(2):
                        nc.sync.dma_start(
                            tp[:, half * S : (half + 1) * S],
                            supT[p * 256 + half * P : p * 256 + (half + 1) * P, :],
                        )

                # ---------------- setup: t first, then s / bigH ----------------
                # t'_bc[p, j] = (W_map V')^T featTs = wv^T featTs, already
                # broadcast to all 128 partitions: lhsT = wv replicated.
                wvB = stat.tile([DIN, P], bf16, tag="wvB")
                nc.vector.tensor_copy(wvB[:], wv_sb[:].broadcast_to([DIN, P]))
                t_bc = stat.tile([P, S], bf16, tag="t_bc")
                for half in range(2):
                    ps_b = psx.tile([P, 512], f32, tag="ps")
                    nc.tensor.matmul(
                        ps_b[:], wvB[:], fsh[:, half * 512 : (half + 1) * 512],
                        start=True, stop=True,
                    )
                    nc.vector.tensor_copy(t_bc[:, half * 512 : (half + 1) * 512], ps_b[:])

                # s' and h-tiles, chunk by chunk; s' in 8 small tiles so the
                # main loop can start as soon as the first chunk is done.
                sb_tiles = []
                for g in range(8):
                    fch = f_chunks[g]
                    # s' column per c-tile: featT_chunk^T @ wu
                    ps_s = psx.tile([P, 8], f32, tag="ps")
                    for k in range(8):
                        nc.tensor.matmul(
                            ps_s[:, k : k + 1],
                            fch[:, k * P : (k + 1) * P],
                            wu_sb[:],
                            start=True,
                            stop=True,
                        )
                    # s_b = s' + (A*b_map + B): per-partition sigmoid bias
                    s_g = stat.tile([P, 8], f32, tag=f"s_{g}")
                    sb_tiles.append(s_g)
                    nc.scalar.activation(
                        s_g[:], ps_s[:], Act.Identity, bias=b_bc[:], scale=1.0
                    )
                    # bigH h-chunks: batch the 8 PSUM->SBUF copies into one
                    ps_bh = psx.tile([P, 8 * DG], f32, tag="ps")
                    for k in range(8):
                        nc.tensor.matmul(
                            ps_bh[:, k * DG : (k + 1) * DG],
                            fch[:, k * P : (k + 1) * P],
                            wmap_sb[:],
                            start=True,
                            stop=True,
                        )
                    nc.vector.tensor_copy(
                        bigH[:, g * 8 * (DG + 1) : (g + 1) * 8 * (DG + 1)]
                        .rearrange("p (ct w) -> p ct w", w=DG + 1)[:, :, 0:DG],
                        ps_bh[:].rearrange("p (ct w) -> p ct w", w=DG),
                    )


                # PSUM accumulators (1 bank each)
                un0 = ps_acc.tile([DG + 1, 512], f32, tag="un0")
                un1 = ps_acc.tile([DG + 1, 512], f32, tag="un1")
                unnorm = (un0, un1)
                uc0 = ps_acc.tile([DG + 1, 512], f32, tag="uc0")
                uc1 = ps_acc.tile([DG + 1, 512], f32, tag="uc1")
                ucorr = (uc0, uc1)
                m0 = ps_acc.tile([DG, 512], f32, tag="m0")
                m1 = ps_acc.tile([DG, 512], f32, tag="m1")
                mm = (m0, m1)

                # ---------------- phase 1 main loop ----------------
                for p in ([] if skip_main else range(NPAIR)):
                    tp = t_tiles[p]
                    g = ring.tile([P, F2], bf16, tag="g")
                    for half in range(2):
                        ct = 2 * p + half
                        nc.scalar.activation(
                            g[:, half * S : (half + 1) * S], t_bc[:],
                            Act.Sigmoid, bias=sb_tiles[ct // 8][:, ct % 8 : ct % 8 + 1],
                            scale=1.0,
                        )
                    mc = ring.tile([P, F2], bf16, tag="mc")
                    nc.vector.tensor_scalar(mc[:], tp[:], 0.0, 2.0, Alu.is_gt, Alu.mult)
                    n = ring.tile([P, F2], bf16, tag="n")
                    nc.vector.tensor_tensor(n[:], mc[:], g[:], Alu.min)
                    for half in range(2):
                        ct = 2 * p + half
                        lhs = bigH[:, ct * (DG + 1) : (ct + 1) * (DG + 1)]
                        for jb in range(2):
                            sl = slice(half * S + jb * 512, half * S + (jb + 1) * 512)
                            nc.tensor.matmul(
                                unnorm[jb][:], lhs, n[:, sl],
                                start=(p == 0 and half == 0),
                                stop=(p == NPAIR - 1 and half == 1),
                            )
                            # same stationary, moving = mask*2: accumulates
                            # 2*sum(mask*[h|1]); combined as +D/2 in the tail
                            nc.tensor.matmul(
                                ucorr[jb][:], lhs, mc[:, sl],
                                start=(p == 0 and half == 0),
                                stop=(p == NPAIR - 1 and half == 1),
                            )

                if skip_tail:
                    zz = tail8.tile([P, DG], f32, tag='fin')
                    nc.vector.memset(zz[:], 0.0)
                    for q in range(8):
                        nc.sync.dma_start(out_sh[q * P : (q + 1) * P, :], zz[:])
                else:
                    # ---------------- tail: gat, all-gather -----------------------
                    # comb = unnorm + (D/2) * ucorr  (the +D offset term);
                    # stt may read only one PSUM operand, so stage ucorr in SBUF
                    ucsb = ring.tile([DG + 1, S], f32, tag="n")
                    for jb in range(2):
                        nc.vector.tensor_copy(
                            ucsb[:, jb * 512 : (jb + 1) * 512], ucorr[jb][:]
                        )
                    comb = ring.tile([DG + 1, S], f32, tag="g")
                    for jb in range(2):
                        nc.vector.scalar_tensor_tensor(
                            comb[:, jb * 512 : (jb + 1) * 512],
                            ucsb[:, jb * 512 : (jb + 1) * 512], SIG_D / 2.0,
                            unnorm[jb][:],
                            Alu.mult, Alu.add,
                        )
                    d_sb = stat.tile([1, S], f32, tag="d_sb")
                    nc.scalar.activation(d_sb[:], comb[DG : DG + 1, :], Act.Copy)
                    rec = ring.tile([DG, S], f32, tag="n")
                    for jb in range(2):
                        ps_d = psx.tile([DG, 512], f32, tag="ps")
                        nc.tensor.matmul(
                            ps_d[:], ones_row[:, 0:DG],
                            d_sb[:, jb * 512 : (jb + 1) * 512],
                            start=True, stop=True,
                        )
                        nc.vector.reciprocal(rec[:, jb * 512 : (jb + 1) * 512], ps_d[:])
                    gv = ring.tile([DG, S], f32, tag="mc")
                    nc.vector.tensor_mul(gv[:], comb[0:DG, :], rec[:])
                    gatT = stat.tile([DG, S], bf16, tag="gatT")
                    nc.scalar.activation(gatT[:], gv[:], Act.Tanh)
                    # transpose gatT -> gat natural [1024, 64]; one batched
                    # store, ONE collective (fixed cost dominates), two
                    # ct-major reloads.
                    gn = stat.tile([P, 8 * DG], fp8, tag="gn")
                    for q in range(8):
                        ps_g = psx.tile([P, DG], bf16, tag="ps")
                        nc.tensor.transpose(
                            ps_g[:], gatT[:, q * P : (q + 1) * P],
                            ident_b[0:DG, 0:DG],
                        )
                        nc.vector.tensor_copy(
                            gn[:, q * DG : (q + 1) * DG], ps_g[:]
                        )
                    nc.sync.dma_start(
                        gat_in[:].rearrange("(q p d) -> p q d", q=8, p=P),
                        gn[:].rearrange("p (q d) -> p q d", d=DG),
                    )
                    nc.gpsimd.collective_compute(
                        "AllGather",
                        Alu.bypass,
                        replica_groups=[list(range(M_CORES))],
                        ins=[gat_in[:]],
                        outs=[gat_all[:]],
                    )
                    gat_sb = []
                    for hh in range(2):
                        # reuse a dead featT-chunk buffer (same byte size)
                        gsb8 = setup.tile([P, 32 * DG], fp8, tag=f"fg{hh}", bufs=1)
                        nc.sync.dma_start(
                            gsb8[:].rearrange("p (ct d) -> p ct d", d=DG),
                            gat_all[
                                hh * 32 * P * DG : (hh + 1) * 32 * P * DG
                            ].rearrange("(ct p d) -> p ct d", p=P, d=DG),
                        )
                        gsb = setup.tile([P, 32 * DG], bf16, tag="fch")
                        gat_sb.append(gsb)
                        nc.vector.tensor_copy(gsb[:], gsb8[:])

                    if skip_p2:
                        zz2 = tail8.tile([P, DG], f32, tag='fin')
                        nc.vector.memset(zz2[:], 0.0)
                        for q in range(8):
                            nc.sync.dma_start(out_sh[q * P : (q + 1) * P, :], zz2[:])
                    else:
                        # ---------------- phase 2 (jb-outer: jb=0's post-chain
                        # overlaps jb=1's matmul sweep) ----------------
                        m_sb = ring.tile([DG, S], f32, tag="mc")
                        reluT = ring.tile([DG, S], f32, tag="n")
                        onats = [None] * 8
                        n2_all = stat.tile([P, 8], f32, tag="n2_all")
                        sqs = stat.tile([P, DG], f32, tag="sqs")
                        for jb in range(2):
                            first = True
                            for hh in range(2):
                                for rank in range(8):
                                    for k in range(4):
                                        ct = hh * 32 + rank * 4 + k
                                        p_idx, half = ct // 2, ct % 2
                                        lhs = gat_sb[hh][
                                            :, (rank * 4 + k) * DG : (rank * 4 + k + 1) * DG
                                        ]
                                        nc.tensor.matmul(
                                            mm[jb][:],
                                            lhs,
                                            t_tiles[p_idx][
                                                :,
                                                half * S + jb * 512 : half * S
                                                + (jb + 1) * 512,
                                            ],
                                            start=first,
                                            stop=(hh == 1 and rank == 7 and k == 3),
                                        )
                                        first = False
                            nc.vector.tensor_copy(
                                m_sb[:, jb * 512 : (jb + 1) * 512], mm[jb][:]
                            )
                            ps_o = psx.tile([DG, 512], f32, tag="ps")
                            nc.tensor.matmul(
                                ps_o[:], wgcn_sb[:], m_sb[:, jb * 512 : (jb + 1) * 512],
                                start=True, stop=True,
                            )
                            nc.scalar.activation(
                                reluT[:, jb * 512 : (jb + 1) * 512], ps_o[:], Act.Relu
                            )
                            for q in range(jb * 4, jb * 4 + 4):
                                ps_t2 = psx.tile([P, DG], f32, tag="ps")
                                nc.tensor.transpose(
                                    ps_t2[:], reluT[:, q * P : (q + 1) * P],
                                    ident_f[0:DG, 0:DG],
                                )
                                onat = tail8.tile([P, DG], f32, tag="onat")
                                nc.vector.tensor_copy(onat[:], ps_t2[:])
                                onats[q] = onat
                                nc.scalar.activation(
                                    sqs[:], ps_t2[:], Act.Square,
                                    accum_out=n2_all[:, q : q + 1],
                                )

                        # ---------------- normalize + store ----------------
                        nrm = stat.tile([P, 8], f32, tag="nrm")
                        nc.scalar.activation(nrm[:], n2_all[:], Act.Sqrt)
                        nc.vector.tensor_scalar_max(nrm[:], nrm[:], 1e-12)
                        rcl = stat.tile([P, 8], f32, tag="rcl")
                        nc.vector.reciprocal(rcl[:], nrm[:])
                        for q in range(8):
                            fin = tail8.tile([P, DG], f32, tag="fin")
                            nc.vector.tensor_scalar_mul(fin[:], onats[q][:], rcl[:, q : q + 1])
                            nc.sync.dma_start(out_sh[q * P : (q + 1) * P, :], fin[:])

    if not nc.is_finalized():
        nc.finalize()
    return nc


def _get_nc(reps=1):
    if reps not in _built:
        _built[reps] = _build(reps)
    return _built[reps]


def _make_in_maps(feat, sup, W_map, b_map, U, V, W_gcn):
    import ml_dtypes

    bf = ml_dtypes.bfloat16
    feat = np.ascontiguousarray(np.asarray(feat, dtype=np.float32))
    sup = np.asarray(sup, dtype=np.float32)
    W_map_f = np.asarray(W_map, dtype=np.float32)
    W_map_np = np.ascontiguousarray(W_map_f).astype(bf)
    wu_np = np.ascontiguousarray(
        W_map_f @ (SIG_A * np.asarray(U, dtype=np.float32))
    ).astype(bf)
    wv_np = np.ascontiguousarray(
        W_map_f @ (SIG_A * np.asarray(V, dtype=np.float32))
    ).astype(bf)
    b_np = np.asarray(
        SIG_A * np.asarray(b_map, dtype=np.float32).reshape(1) + SIG_B,
        dtype=np.float32,
    )
    W_gcn_np = np.ascontiguousarray(np.asarray(W_gcn, dtype=np.float32))

    featT = np.ascontiguousarray(feat.T).astype(bf)
    idx = np.arange(S)
    in_maps = []
    for r in range(M_CORES):
        shard = np.array(sup[r * S : (r + 1) * S, :], dtype=np.float32, copy=True)
        shard[idx, r * S + idx] += 1.0  # self loops
        in_maps.append(
            {
                "supT": np.ascontiguousarray(shard.T).astype(bf),
                "featT": featT,
                "featTs": np.ascontiguousarray(featT[:, r * S : (r + 1) * S]),
                "W_map": W_map_np,
                "wu": wu_np,
                "wv": wv_np,
                "bfit": b_np,
                "W_gcn": W_gcn_np,
            }
        )
    return in_maps


def kernel(feat, sup, W_map, b_map, U, V, W_gcn):
    from concourse.bass_utils import run_bass_kernel_spmd

    in_maps = _make_in_maps(feat, sup, W_map, b_map, U, V, W_gcn)
    nc = _get_nc()
    trace = bool(int(os.environ.get("KERNEL_TRACE", "0")))
    try:
        res = run_bass_kernel_spmd(
            nc, in_maps, core_ids=list(range(M_CORES)), trace=trace,
            stitch_traces=False,
        )
    except Exception:
        if not trace:
            raise
        res = run_bass_kernel_spmd(
            nc, in_maps, core_ids=list(range(M_CORES)), trace=False,
            stitch_traces=False,
        )
    if trace and res.exec_time_ns is not None:
        print(f"HW exec time: {res.exec_time_ns} ns")
        kernel.last_exec_time_ns = res.exec_time_ns
        kernel.last_results = res
    out = np.concatenate(
        [res.results[r]["out_shard"] for r in range(M_CORES)], axis=0
    )
    return out.astype(np.float32)
